# revision 9
# baseline (speedup 1.0000x reference)
"""BatchAllTripletLoss on 8 Trainium2 NeuronCores.

Strategy
--------
The loss  sum_{i,j,k} relu(d(i,j) - d(i,k) + m) * mask / (count + eps)  is
invariant to a permutation of the batch, so on the host we sort the batch by
label.  After sorting every class is one contiguous slice, which turns the
label masks into compile-time column slices.

Each core gets 64 uniform "slot" anchors (each class is split evenly over the
8 cores so the compiled program is identical on every core - required for
SPMD) plus <=1 "dense" leftover anchor.  On device, per core:

  1. normalize embeddings via squares + ones-matmul column sums
  2. G = Xanch @ X^T (fp32 PE matmul), D = 1 - G * invn_i * invn_j
  3. POS[i,q] = D[i, class-slice]+m (compacted, diag/dummy masked to -1e9)
     NEG[i,k] = D[i,k] masked to +1e9 at same-class columns
  4. main loop over stacked bias columns (2 positives per instruction using
     all 128 partitions): ScalarE does relu(bias - NEG) with free-dim
     accumulation, VectorE counts (NEG < bias) with accumulation
  5. leftover anchors use a broadcast-row dense pass (4 instrs each)
  6. per-core [sum, count] partials reduced by a ones-matmul, host divides

All mask logic is folded into +-1e9 sentinel values so no triplet tensor is
ever materialized.
"""

import numpy as np

B, D, NCORES = 512, 768, 8
MARGIN = 0.5
EPS = 1e-8
BIG = 1e9

_PROG_CACHE: dict = {}


# --------------------------------------------------------------------------
# host-side plan (pure numpy, derived from labels only)
# --------------------------------------------------------------------------
class Plan:
    pass


def _make_plan(labels: np.ndarray) -> Plan:
    p = Plan()
    order = np.argsort(labels, kind="stable")
    lab = labels[order]
    nclass = int(lab.max()) + 1
    counts = np.bincount(lab, minlength=nclass).astype(int)
    # drop empty classes from the slice list
    ks = [k for k in range(nclass) if counts[k] > 0]
    n = [int(counts[k]) for k in ks]
    starts = np.concatenate([[0], np.cumsum(n)]).astype(int)

    # uniform slots per class: m_k slots on every core; shave until sum<=64
    m = [int(np.ceil(nk / NCORES)) for nk in n]
    while sum(m) > 64:
        # shaving class k moves (nk - 8*(mk-1)) anchors to the leftover pool
        best = min(
            (i for i in range(len(m)) if m[i] > 0),
            key=lambda i: n[i] - NCORES * (m[i] - 1),
        )
        m[best] -= 1
    Mslots = 64  # pad slot rows to exactly 64 so dense rows start at partition 64
    offs = np.concatenate([[0], np.cumsum(m)]).astype(int)

    leftovers = []  # (class_idx_in_ks, within_class_w)
    for i in range(len(n)):
        for w in range(NCORES * m[i], n[i]):
            leftovers.append((i, w))
    Ld = int(np.ceil(len(leftovers) / NCORES)) if leftovers else 0

    Kpos = max(n)
    Kpos2 = Kpos + (Kpos % 2)  # pad to even for 2x stacking
    J2 = Kpos2 // 2
    R = Mslots + Ld

    # per-core tables
    anch_idx = np.zeros((NCORES, R), dtype=np.int64)  # sorted-order index
    posmask = np.zeros((NCORES, Mslots, Kpos2), dtype=np.int8)
    negmask = np.zeros((NCORES, Mslots, B), dtype=np.int8)
    pmd = np.zeros((NCORES, max(Ld, 1), B), dtype=np.int8)
    nmd = np.zeros((NCORES, max(Ld, 1), B), dtype=np.int8)
    for c in range(NCORES):
        for i in range(len(n)):
            s, nk, mk, off = starts[i], n[i], m[i], offs[i]
            for t in range(mk):
                w = NCORES * t + c
                r = off + t
                if w < nk:
                    anch_idx[c, r] = s + w
                    posmask[c, r, :nk] = 1
                    posmask[c, r, w] = 0  # exclude j == i
                    negmask[c, r, :] = 1
                    negmask[c, r, s : s + nk] = 0
                else:
                    anch_idx[c, r] = s  # dummy slot, fully masked
        for dr in range(Ld):
            li = c + NCORES * dr
            if li < len(leftovers):
                i, w = leftovers[li]
                s, nk = starts[i], n[i]
                a = s + w
                anch_idx[c, Mslots + dr] = a
                pmd[c, dr, s : s + nk] = 1
                pmd[c, dr, a] = 0
                nmd[c, dr, :] = 1
                nmd[c, dr, s : s + nk] = 0
            else:
                anch_idx[c, Mslots + dr] = 0  # dummy, masks stay 0

    p.order = order
    p.n = n
    p.starts = starts
    p.m = m
    p.offs = offs
    p.Mslots = Mslots
    p.Ld = Ld
    p.Kpos = Kpos
    p.Kpos2 = Kpos2
    p.J2 = J2
    p.R = R
    p.anch_idx = anch_idx
    p.posmask = posmask
    p.negmask = negmask
    # per-class row-restricted positive masks (for full-height predicated
    # copies: engine APs cannot start at arbitrary partitions)
    pm7 = np.zeros((NCORES, len(n), Mslots, Kpos2), dtype=np.int8)
    for c in range(NCORES):
        for i in range(len(n)):
            rows = slice(offs[i], offs[i] + m[i])
            pm7[c, i, rows, :] = posmask[c, rows, :]
    p.pm7 = pm7
    p.pmd = pmd
    p.nmd = nmd
    p.key = (tuple(n), tuple(m), Ld)
    return p


# --------------------------------------------------------------------------
# device program
# --------------------------------------------------------------------------
def _build_program(p: Plan):
    from contextlib import ExitStack

    import concourse.bacc as bacc
    import concourse.mybir as mybir
    import concourse.tile as tile

    f32 = mybir.dt.float32
    Alu = mybir.AluOpType
    Act = mybir.ActivationFunctionType
    X = mybir.AxisListType.X

    M, Ld, R, J2, Kpos2 = p.Mslots, p.Ld, p.R, p.J2, p.Kpos2
    NCLS = len(p.n)

    nc = bacc.Bacc("TRN2", target_bir_lowering=False, debug=False, num_devices=NCORES)

    xT = nc.dram_tensor("xT", [D, B], f32, kind="ExternalInput").ap()
    xaT = nc.dram_tensor("xaT", [D, R], f32, kind="ExternalInput").ap()
    xa = nc.dram_tensor("xa", [R, D], f32, kind="ExternalInput").ap()
    i8 = mybir.dt.int8
    pm7 = nc.dram_tensor("pm7", [NCLS, M, Kpos2], i8, kind="ExternalInput").ap()
    nm = nc.dram_tensor("nm", [M, B], i8, kind="ExternalInput").ap()
    if Ld:
        pmdd = nc.dram_tensor("pmd", [Ld, B], mybir.dt.int8, kind="ExternalInput").ap()
        nmdd = nc.dram_tensor("nmd", [Ld, B], mybir.dt.int8, kind="ExternalInput").ap()
        eye = nc.dram_tensor("eye", [Ld, Ld], f32, kind="ExternalInput").ap()
    out = nc.dram_tensor("out", [1, 4], f32, kind="ExternalOutput").ap()

    NCH = D // 128  # 6 contraction chunks

    with tile.TileContext(nc) as tc, ExitStack() as ctx:
        pool = ctx.enter_context(tc.tile_pool(name="sb", bufs=1))
        sqpool = ctx.enter_context(tc.tile_pool(name="sq", bufs=2))
        scrA = ctx.enter_context(tc.tile_pool(name="scrA", bufs=2))
        scrV = ctx.enter_context(tc.tile_pool(name="scrV", bufs=2))
        pp = ctx.enter_context(tc.tile_pool(name="ps", bufs=1, space="PSUM"))
        ppnb = ctx.enter_context(tc.tile_pool(name="psnb", bufs=2, space="PSUM"))

        # ---- constants -------------------------------------------------
        ones_col = pool.tile([128, 1], f32)  # lhsT for partition sums
        nc.gpsimd.memset(ones_col[:], 1.0)
        ones_row = pool.tile([1, 128], f32)  # lhsT for broadcasts
        nc.gpsimd.memset(ones_row[:], 1.0)

        # ---- loads -----------------------------------------------------
        xT_t = pool.tile([128, NCH, B], f32)
        nc.sync.dma_start(xT_t[:], xT.rearrange("(c p) j -> p c j", p=128))
        xaT_t = pool.tile([128, NCH, R], f32)
        nc.sync.dma_start(xaT_t[:], xaT.rearrange("(c p) j -> p c j", p=128))
        xa_t = pool.tile([R, D], f32)
        nc.sync.dma_start(xa_t[:], xa)
        pm7_t = pool.tile([M, NCLS, Kpos2], i8)
        nc.sync.dma_start(pm7_t[:], pm7.rearrange("k m q -> m k q"))
        nm_t = pool.tile([M, B], i8)
        nc.sync.dma_start(nm_t[:], nm)
        if Ld:
            pmd_t = pool.tile([Ld, B], i8)
            nc.sync.dma_start(pmd_t[:], pmdd)
            nmd_t = pool.tile([Ld, B], i8)
            nc.sync.dma_start(nmd_t[:], nmdd)
            eye_t = pool.tile([Ld, Ld], f32)
            nc.sync.dma_start(eye_t[:], eye)

        # ---- column norms of X: ssq[j] = sum_d xT[d,j]^2 ---------------
        ps_ssq = pp.tile([1, B], f32)
        for q in range(NCH):
            sq = sqpool.tile([128, B], f32, tag="sq")
            nc.scalar.activation(sq[:], xT_t[:, q, :], Act.Square)
            nc.tensor.matmul(
                ps_ssq[:], ones_col[:], sq[:], start=(q == 0), stop=(q == NCH - 1)
            )
        nrm = pool.tile([1, B], f32)
        nc.scalar.activation(nrm[:], ps_ssq[:], Act.Sqrt)
        invn = pool.tile([1, B], f32)
        nc.vector.reciprocal(invn[:], nrm[:])

        # ---- anchor norms ----------------------------------------------
        scr_a = pool.tile([R, D], f32)
        ssqa = pool.tile([R, 1], f32)
        nc.scalar.activation(scr_a[:], xa_t[:], Act.Square, accum_out=ssqa[:])
        nrma = pool.tile([R, 1], f32)
        nc.scalar.activation(nrma[:], ssqa[:], Act.Sqrt)
        invna = pool.tile([R, 1], f32)
        nc.vector.reciprocal(invna[:], nrma[:])
        invna_n = pool.tile([R, 1], f32)
        nc.vector.tensor_scalar_mul(invna_n[:], invna[:], -1.0)

        # ---- G = Xanch @ X^T ; D = 1 - G*invna*invn --------------------
        ps_G = pp.tile([R, B], f32)
        for q in range(NCH):
            nc.tensor.matmul(
                ps_G[:], xaT_t[:, q, :], xT_t[:, q, :],
                start=(q == 0), stop=(q == NCH - 1),
            )
        ps_B = pp.tile([R, B], f32)
        nc.tensor.matmul(ps_B[:], ones_row[:, :R], invn[:], start=True, stop=True)
        invnB = pool.tile([R, B], f32)
        nc.scalar.activation(invnB[:], ps_B[:], Act.Copy)
        t1 = pool.tile([R, B], f32)
        nc.vector.tensor_tensor(t1[:], ps_G[:], invnB[:], Alu.mult)
        Dm = pool.tile([R, B], f32)
        nc.vector.tensor_scalar(Dm[:], t1[:], invna_n[:], 1.0, Alu.mult, Alu.add)

        # ---- POS (compacted via full-height predicated copies; the margin
        # is folded into NEG as (d_ik - m), so POS carries plain d values) --
        posf = pool.tile([M, Kpos2], f32)
        nc.gpsimd.memset(posf[:], -BIG)
        for i in range(len(p.n)):
            s, nk, mk = p.starts[i], p.n[i], p.m[i]
            if mk == 0:
                continue
            nc.vector.copy_predicated(
                posf[:, 0:nk], pm7_t[:, i, 0:nk], Dm[0:M, s : s + nk]
            )

        # stacked bias columns: rows [0,M) even j, rows [64,64+M) odd j
        POSst = pool.tile([128, J2], f32)
        nc.gpsimd.memset(POSst[:], -BIG)
        pe = posf.rearrange("p (a two) -> p two a", two=2)
        nc.vector.tensor_copy(POSst[0:M, :], pe[:, 0, :])
        nc.sync.dma_start(POSst[64 : 64 + M, :], pe[:, 1, :])

        # ---- NEG (dense, host mask; same-class columns stay +BIG) -------
        NEGst = pool.tile([128, B], f32)
        nc.gpsimd.memset(NEGst[:], BIG)
        nc.vector.copy_predicated(NEGst[0:M, :], nm_t[:], Dm[0:M, :])
        # engines cannot shift partitions; duplicate the lower half via DMA
        nc.sync.dma_start(NEGst[64 : 64 + M, :], NEGst[0:M, :])
        NEGm = pool.tile([128, B], f32)
        nc.vector.tensor_scalar_add(NEGm[:], NEGst[:], -MARGIN)

        # dummy slots must not contribute: their posmask rows are all zero,
        # so their bias is -BIG and every term is already 0. (NEG rows of
        # dummies hold real data; harmless.)

        # ---- main loop --------------------------------------------------
        sumcols = pool.tile([128, J2], f32)
        cntcols = pool.tile([128, J2], f32)
        for jj in range(J2):
            sA = scrA.tile([128, B], f32, tag="sA")
            nc.scalar.activation(
                sA[:], NEGm[:], Act.Relu,
                bias=POSst[:, jj : jj + 1], scale=-1.0,
                accum_out=sumcols[:, jj : jj + 1],
            )
            sV = scrV.tile([128, B], f32, tag="sV")
            nc.vector.tensor_scalar(
                sV[:], NEGm[:], POSst[:, jj : jj + 1], None,
                Alu.is_lt, Alu.add, accum_out=cntcols[:, jj : jj + 1],
            )

        # ---- dense pass for leftover anchors ---------------------------
        if Ld:
            # bounce the dense D rows down to base partition 0 (predicated
            # copies need equal base partitions for all operands)
            Dmd = pool.tile([Ld, B], f32)
            nc.sync.dma_start(Dmd[:], Dm[M:R, :])
            posd = pool.tile([Ld, B], f32)
            nc.gpsimd.memset(posd[:], -BIG)
            nc.vector.copy_predicated(posd[:], pmd_t[:], Dmd[:])
            negd = pool.tile([Ld, B], f32)
            nc.gpsimd.memset(negd[:], BIG)
            nc.vector.copy_predicated(negd[:], nmd_t[:], Dmd[:])
            negdm = pool.tile([Ld, B], f32)
            nc.vector.tensor_scalar_add(negdm[:], negd[:], -MARGIN)

            NQ = B // 128
            posdT = pool.tile([128, NQ * Ld], f32)
            for q in range(NQ):
                ps_t = ppnb.tile([128, Ld], f32, tag="pst")
                nc.tensor.matmul(
                    ps_t[:], posd[:, q * 128 : (q + 1) * 128], eye_t[:],
                    start=True, stop=True,
                )
                nc.scalar.activation(
                    posdT[:, q * Ld : (q + 1) * Ld], ps_t[:], Act.Copy
                )

            sumd = pool.tile([128, NQ * Ld], f32)
            cntd = pool.tile([128, NQ * Ld], f32)
            for dr in range(Ld):
                if dr == 0:
                    row_ap = negdm[0:1, :]
                else:
                    # engine APs cannot start at partition dr>0; bounce via DMA
                    row = pool.tile([1, B], f32, tag=f"negrow{dr}")
                    nc.sync.dma_start(row[:], negdm[dr : dr + 1, :])
                    row_ap = row[:]
                ps_nb = ppnb.tile([128, B], f32, tag="nb")
                nc.tensor.matmul(
                    ps_nb[:], ones_row[:], row_ap, start=True, stop=True,
                )
                for q in range(NQ):
                    col = q * Ld + dr
                    sA = scrA.tile([128, B], f32, tag="sA")
                    nc.scalar.activation(
                        sA[:], ps_nb[:], Act.Relu,
                        bias=posdT[:, col : col + 1], scale=-1.0,
                        accum_out=sumd[:, col : col + 1],
                    )
                    sV = scrV.tile([128, B], f32, tag="sV")
                    nc.vector.tensor_scalar(
                        sV[:], ps_nb[:], posdT[:, col : col + 1], None,
                        Alu.is_lt, Alu.add, accum_out=cntd[:, col : col + 1],
                    )

        # ---- final reduction -------------------------------------------
        V = pool.tile([128, 4], f32)
        nc.gpsimd.memset(V[:], 0.0)
        nc.vector.tensor_reduce(V[:, 0:1], sumcols[:], X, Alu.add)
        nc.vector.tensor_reduce(V[:, 1:2], cntcols[:], X, Alu.add)
        if Ld:
            nc.vector.tensor_reduce(V[:, 2:3], sumd[:], X, Alu.add)
            nc.vector.tensor_reduce(V[:, 3:4], cntd[:], X, Alu.add)
        ps_f = pp.tile([1, 4], f32)
        nc.tensor.matmul(ps_f[:], ones_col[:], V[:], start=True, stop=True)
        outs = pool.tile([1, 4], f32)
        nc.scalar.activation(outs[:], ps_f[:], Act.Copy)
        nc.sync.dma_start(out, outs[:])

    nc.compile()
    return nc


def _in_maps(p: Plan, emb: np.ndarray):
    xs = np.ascontiguousarray(emb[p.order])  # sorted by label
    xT = np.ascontiguousarray(xs.T)
    maps = []
    for c in range(NCORES):
        xa = np.ascontiguousarray(xs[p.anch_idx[c]])
        m = {
            "xT": xT,
            "xaT": np.ascontiguousarray(xa.T),
            "xa": xa,
            "pm7": p.pm7[c],
            "nm": p.negmask[c],
        }
        if p.Ld:
            m["pmd"] = p.pmd[c]
            m["nmd"] = p.nmd[c]
            m["eye"] = np.eye(p.Ld, dtype=np.float32)
        maps.append(m)
    return maps


LAST_RESULT = None  # BassKernelResults of the most recent run (for profiling)


def kernel(embeddings, labels):
    global LAST_RESULT
    import os

    from concourse.bass_utils import run_bass_kernel_spmd

    emb = np.ascontiguousarray(np.asarray(embeddings, dtype=np.float32))
    lab = np.asarray(labels).astype(np.int64)
    p = _make_plan(lab)
    if p.key not in _PROG_CACHE:
        _PROG_CACHE[p.key] = _build_program(p)
    nc = _PROG_CACHE[p.key]
    trace = bool(int(os.environ.get("TRIPLET_TRACE", "0")))
    kw = {}
    if os.environ.get("TRIPLET_TMPDIR"):
        kw["tmpdir"] = os.environ["TRIPLET_TMPDIR"]
    LAST_RESULT = run_bass_kernel_spmd(
        nc, _in_maps(p, emb), list(range(NCORES)), trace=trace, **kw
    )
    res = LAST_RESULT.results
    S = 0.0
    C = 0.0
    for r in res:
        o = np.asarray(r["out"], dtype=np.float64).reshape(-1)
        S += o[0] + o[2]
        C += o[1] + o[3]
    return np.float32(S / (C + EPS))


# revision 10
# speedup vs baseline: 1.2733x; 1.2733x over previous
"""BatchAllTripletLoss on 8 Trainium2 NeuronCores.

Strategy
-------
The loss  sum_{i,j,k} relu(d(i,j) - d(i,k) + m) * mask / (count + eps)  is
invariant to batch permutation, so the host sorts the batch by label; every
class becomes one contiguous column slice.  Core c owns the 64 sorted anchors
[64c, 64c+64).  All mask logic (class membership, j!=i diagonal) is carried
by per-core int8 mask tensors, so one compiled SPMD program serves all cores.

Per core, on device:
  1. column norms via Square + ones-matmul (bf16 inputs, f32 accumulate)
  2. G = Xanch @ X^T (bf16 PE matmul), D = 1 - G * invn_i * invn_j
  3. POS[i,q] = D[i, class_slice(i)] compacted by per-class predicated
     copies; NEG[i,k] = D[i,k] - margin with same-class columns -> +1e9
     (margin folded into NEG so POS bias needs no add)
  4. main loop over stacked bias columns (each anchor appears twice, on
     partitions p and p+64, taking even/odd positives -> all 128 lanes):
     ScalarE: relu(bias - NEG) with free-dim accumulation
     VectorE: count(NEG < bias) with free-dim accumulation
  5. per-core [sum, count] partials via ones-matmul; host sums and divides

The B^3 triplet tensor is never materialized; the main loop touches
64*88*512 = 2.9M elements per core per pass.
"""

import numpy as np

B, D, NCORES = 512, 768, 8
MA = 64  # anchors per core
MARGIN = 0.5
EPS = 1e-8
BIG = 1e9

_PROG_CACHE: dict = {}


class Plan:
    pass


def _make_plan(labels: np.ndarray) -> Plan:
    p = Plan()
    order = np.argsort(labels, kind="stable")
    lab = labels[order]
    nclass = int(lab.max()) + 1
    counts = np.bincount(lab, minlength=nclass).astype(int)
    n = [int(c) for c in counts if c > 0]
    starts = np.concatenate([[0], np.cumsum(n)]).astype(int)
    cls_of = np.searchsorted(starts, np.arange(B), side="right") - 1

    Kpos = max(n)
    Kpos2 = Kpos + (Kpos % 2)
    J2 = Kpos2 // 2

    posmask = np.zeros((NCORES, MA, Kpos2), dtype=np.int8)
    negmask = np.zeros((NCORES, MA, B), dtype=np.int8)
    pm7 = np.zeros((NCORES, len(n), MA, Kpos2), dtype=np.int8)
    for c in range(NCORES):
        for r in range(MA):
            a = MA * c + r
            i = cls_of[a]
            s, nk = starts[i], n[i]
            posmask[c, r, :nk] = 1
            posmask[c, r, a - s] = 0  # j == i
            negmask[c, r, :] = 1
            negmask[c, r, s : s + nk] = 0
            pm7[c, i, r, :] = posmask[c, r, :]

    p.order = order
    p.n = n
    p.starts = starts
    p.Kpos2 = Kpos2
    p.J2 = J2
    p.posmask = posmask
    p.negmask = negmask
    p.pm7 = pm7
    p.key = tuple(n)
    return p


def _build_program(p: Plan):
    from contextlib import ExitStack

    import concourse.bacc as bacc
    import concourse.mybir as mybir
    import concourse.tile as tile

    f32 = mybir.dt.float32
    bf16 = mybir.dt.bfloat16
    i8 = mybir.dt.int8
    Alu = mybir.AluOpType
    Act = mybir.ActivationFunctionType
    X = mybir.AxisListType.X

    J2, Kpos2 = p.J2, p.Kpos2
    NCLS = len(p.n)
    NCH = D // 128

    nc = bacc.Bacc("TRN2", target_bir_lowering=False, debug=False, num_devices=NCORES)

    xT = nc.dram_tensor("xT", [D, B], bf16, kind="ExternalInput").ap()
    xaT = nc.dram_tensor("xaT", [D, MA], bf16, kind="ExternalInput").ap()
    xa = nc.dram_tensor("xa", [MA, D], bf16, kind="ExternalInput").ap()
    pm7 = nc.dram_tensor("pm7", [NCLS, MA, Kpos2], i8, kind="ExternalInput").ap()
    nm = nc.dram_tensor("nm", [MA, B], i8, kind="ExternalInput").ap()
    out = nc.dram_tensor("out", [1, 2], f32, kind="ExternalOutput").ap()

    with tile.TileContext(nc) as tc, ExitStack() as ctx:
        pool = ctx.enter_context(tc.tile_pool(name="sb", bufs=1))
        sqpool = ctx.enter_context(tc.tile_pool(name="sq", bufs=3))
        scrA = ctx.enter_context(tc.tile_pool(name="scrA", bufs=2))
        scrV = ctx.enter_context(tc.tile_pool(name="scrV", bufs=2))
        pp = ctx.enter_context(tc.tile_pool(name="ps", bufs=1, space="PSUM"))

        ones_bf = pool.tile([128, 1], bf16)
        nc.gpsimd.memset(ones_bf[:], 1.0)
        ones_f32 = pool.tile([128, 1], f32)
        nc.gpsimd.memset(ones_f32[:], 1.0)
        ones_row = pool.tile([1, MA], f32)
        nc.gpsimd.memset(ones_row[:], 1.0)

        # ---- loads (per-chunk so squares/matmuls pipeline) --------------
        xTv = xT.rearrange("(c p) j -> p c j", p=128)
        xT_t = pool.tile([128, NCH, B], bf16)
        for q in range(NCH):
            nc.sync.dma_start(xT_t[:, q, :], xTv[:, q, :])
        xaTv = xaT.rearrange("(c p) j -> p c j", p=128)
        xaT_t = pool.tile([128, NCH, MA], bf16)
        nc.sync.dma_start(xaT_t[:], xaTv)
        xa_t = pool.tile([MA, D], bf16)
        nc.sync.dma_start(xa_t[:], xa)
        pm7_t = pool.tile([MA, NCLS, Kpos2], i8)
        nc.sync.dma_start(pm7_t[:], pm7.rearrange("k m q -> m k q"))
        nm_t = pool.tile([MA, B], i8)
        nc.sync.dma_start(nm_t[:], nm)

        # ---- column norms ssq[j] = sum_d x[d,j]^2 -----------------------
        ps_ssq = pp.tile([1, B], f32)
        for q in range(NCH):
            sq = sqpool.tile([128, B], bf16, tag="sq")
            nc.scalar.activation(sq[:], xT_t[:, q, :], Act.Square)
            nc.tensor.matmul(
                ps_ssq[:], ones_bf[:], sq[:], start=(q == 0), stop=(q == NCH - 1)
            )
        nrm = pool.tile([1, B], f32)
        nc.scalar.activation(nrm[:], ps_ssq[:], Act.Sqrt)
        invn = pool.tile([1, B], f32)
        nc.vector.reciprocal(invn[:], nrm[:])

        # ---- anchor norms ----------------------------------------------
        scr_a = pool.tile([MA, D], bf16)
        ssqa = pool.tile([MA, 1], f32)
        nc.scalar.activation(scr_a[:], xa_t[:], Act.Square, accum_out=ssqa[:])
        nrma = pool.tile([MA, 1], f32)
        nc.scalar.activation(nrma[:], ssqa[:], Act.Sqrt)
        invna = pool.tile([MA, 1], f32)
        nc.vector.reciprocal(invna[:], nrma[:])
        invna_n = pool.tile([MA, 1], f32)
        nc.vector.tensor_scalar_mul(invna_n[:], invna[:], -1.0)

        # ---- G = Xanch @ X^T ; D = 1 - G*invna*invn ---------------------
        ps_G = pp.tile([MA, B], f32)
        for q in range(NCH):
            nc.tensor.matmul(
                ps_G[:], xaT_t[:, q, :], xT_t[:, q, :],
                start=(q == 0), stop=(q == NCH - 1),
            )
        ps_B = pp.tile([MA, B], f32)
        nc.tensor.matmul(ps_B[:], ones_row[:], invn[:], start=True, stop=True)
        invnB = pool.tile([MA, B], f32)
        nc.scalar.activation(invnB[:], ps_B[:], Act.Copy)
        t1 = pool.tile([MA, B], f32)
        nc.vector.tensor_tensor(t1[:], ps_G[:], invnB[:], Alu.mult)
        Dm = pool.tile([MA, B], f32)
        nc.vector.tensor_scalar(Dm[:], t1[:], invna_n[:], 1.0, Alu.mult, Alu.add)

        # ---- POS (compacted, data-driven class assignment) --------------
        posf = pool.tile([MA, Kpos2], f32)
        nc.gpsimd.memset(posf[:], -BIG)
        for i in range(NCLS):
            s, nk = p.starts[i], p.n[i]
            nc.vector.copy_predicated(
                posf[:, 0:nk], pm7_t[:, i, 0:nk], Dm[:, s : s + nk]
            )
        POSst = pool.tile([128, J2], f32)
        nc.gpsimd.memset(POSst[:], -BIG)
        pe = posf.rearrange("p (a two) -> p two a", two=2)
        nc.vector.tensor_copy(POSst[0:MA, :], pe[:, 0, :])
        nc.sync.dma_start(POSst[64 : 64 + MA, :], pe[:, 1, :])

        # ---- NEG (dense, minus margin, bf16) ----------------------------
        NEGf = pool.tile([MA, B], f32)
        nc.gpsimd.memset(NEGf[:], BIG)
        nc.vector.copy_predicated(NEGf[:], nm_t[:], Dm[:])
        NEGm = pool.tile([128, B], bf16)
        nc.vector.tensor_scalar_add(NEGm[0:MA, :], NEGf[:], -MARGIN)
        nc.sync.dma_start(NEGm[64 : 64 + MA, :], NEGm[0:MA, :])

        # ---- main loop ---------------------------------------------------
        sumcols = pool.tile([128, J2], f32)
        cntcols = pool.tile([128, J2], f32)
        for jj in range(J2):
            sA = scrA.tile([128, B], bf16, tag="sA")
            nc.scalar.activation(
                sA[:], NEGm[:], Act.Relu,
                bias=POSst[:, jj : jj + 1], scale=-1.0,
                accum_out=sumcols[:, jj : jj + 1],
            )
            sV = scrV.tile([128, B], bf16, tag="sV")
            nc.vector.tensor_scalar(
                sV[:], NEGm[:], POSst[:, jj : jj + 1], None,
                Alu.is_lt, Alu.add, accum_out=cntcols[:, jj : jj + 1],
            )

        # ---- final reduction --------------------------------------------
        V = pool.tile([128, 2], f32)
        nc.vector.tensor_reduce(V[:, 0:1], sumcols[:], X, Alu.add)
        nc.vector.tensor_reduce(V[:, 1:2], cntcols[:], X, Alu.add)
        ps_f = pp.tile([1, 2], f32)
        nc.tensor.matmul(ps_f[:], ones_f32[:], V[:], start=True, stop=True)
        outs = pool.tile([1, 2], f32)
        nc.scalar.activation(outs[:], ps_f[:], Act.Copy)
        nc.sync.dma_start(out, outs[:])

    nc.compile()
    return nc


def _in_maps(p: Plan, emb: np.ndarray):
    import ml_dtypes

    bf = ml_dtypes.bfloat16
    xs = np.ascontiguousarray(emb[p.order])
    xT = np.ascontiguousarray(xs.T.astype(bf))
    maps = []
    for c in range(NCORES):
        xa = xs[MA * c : MA * (c + 1)]
        maps.append(
            {
                "xT": xT,
                "xaT": np.ascontiguousarray(xa.T.astype(bf)),
                "xa": np.ascontiguousarray(xa.astype(bf)),
                "pm7": p.pm7[c],
                "nm": p.negmask[c],
            }
        )
    return maps


LAST_RESULT = None  # BassKernelResults of the most recent run (for profiling)


def kernel(embeddings, labels):
    global LAST_RESULT
    import os

    from concourse.bass_utils import run_bass_kernel_spmd

    emb = np.ascontiguousarray(np.asarray(embeddings, dtype=np.float32))
    lab = np.asarray(labels).astype(np.int64)
    p = _make_plan(lab)
    if p.key not in _PROG_CACHE:
        _PROG_CACHE[p.key] = _build_program(p)
    nc = _PROG_CACHE[p.key]
    trace = bool(int(os.environ.get("TRIPLET_TRACE", "0")))
    kw = {}
    if os.environ.get("TRIPLET_TMPDIR"):
        kw["tmpdir"] = os.environ["TRIPLET_TMPDIR"]
    LAST_RESULT = run_bass_kernel_spmd(
        nc, _in_maps(p, emb), list(range(NCORES)), trace=trace, **kw
    )
    res = LAST_RESULT.results
    S = 0.0
    C = 0.0
    for r in res:
        o = np.asarray(r["out"], dtype=np.float64).reshape(-1)
        S += o[0]
        C += o[1]
    return np.float32(S / (C + EPS))


# revision 11
# speedup vs baseline: 1.4282x; 1.1217x over previous
"""BatchAllTripletLoss on 8 Trainium2 NeuronCores.

Strategy
-------
The loss  sum_{i,j,k} relu(d(i,j) - d(i,k) + m) * mask / (count + eps)  is
invariant to batch permutation, so the host sorts the batch by label; every
class becomes one contiguous column slice.  Core c owns the 64 sorted anchors
[64c, 64c+64).  All mask logic (class membership, j!=i diagonal) is carried
by per-core int8 mask tensors, so one compiled SPMD program serves all cores.

Per core, on device:
  1. column norms via Square + ones-matmul (bf16 inputs, f32 accumulate)
  2. G = Xanch @ X^T (bf16 PE matmul), D = 1 - G * invn_i * invn_j
  3. POS[i,q] = D[i, class_slice(i)] compacted by per-class predicated
     copies; NEG[i,k] = D[i,k] - margin with same-class columns -> +1e9
     (margin folded into NEG so POS bias needs no add)
  4. main loop over stacked bias columns (each anchor appears twice, on
     partitions p and p+64, taking even/odd positives -> all 128 lanes):
     ScalarE: relu(bias - NEG) with free-dim accumulation
     VectorE: count(NEG < bias) with free-dim accumulation
  5. per-core [sum, count] partials via ones-matmul; host sums and divides

The B^3 triplet tensor is never materialized; the main loop touches
64*88*512 = 2.9M elements per core per pass.
"""

import numpy as np

B, D, NCORES = 512, 768, 8
MA = 64  # anchors per core
MARGIN = 0.5
EPS = 1e-8
BIG = 1e9

_PROG_CACHE: dict = {}


class Plan:
    pass


def _make_plan(labels: np.ndarray) -> Plan:
    p = Plan()
    order = np.argsort(labels, kind="stable")
    lab = labels[order]
    nclass = int(lab.max()) + 1
    counts = np.bincount(lab, minlength=nclass).astype(int)
    n = [int(c) for c in counts if c > 0]
    starts = np.concatenate([[0], np.cumsum(n)]).astype(int)
    cls_of = np.searchsorted(starts, np.arange(B), side="right") - 1

    Kpos = max(n)
    Kpos2 = Kpos + (Kpos % 2)
    J2 = Kpos2 // 2

    posmask = np.zeros((NCORES, MA, Kpos2), dtype=np.int8)
    negmask = np.zeros((NCORES, MA, B), dtype=np.int8)
    pm7 = np.zeros((NCORES, len(n), MA, Kpos2), dtype=np.int8)
    for c in range(NCORES):
        for r in range(MA):
            a = MA * c + r
            i = cls_of[a]
            s, nk = starts[i], n[i]
            posmask[c, r, :nk] = 1
            posmask[c, r, a - s] = 0  # j == i
            negmask[c, r, :] = 1
            negmask[c, r, s : s + nk] = 0
            pm7[c, i, r, :] = posmask[c, r, :]

    p.order = order
    p.n = n
    p.starts = starts
    p.Kpos2 = Kpos2
    p.J2 = J2
    p.posmask = posmask
    p.negmask = negmask
    p.pm7 = pm7
    p.key = tuple(n)
    return p


def _build_program(p: Plan):
    from contextlib import ExitStack

    import concourse.bacc as bacc
    import concourse.mybir as mybir
    import concourse.tile as tile

    f32 = mybir.dt.float32
    bf16 = mybir.dt.bfloat16
    i8 = mybir.dt.int8
    Alu = mybir.AluOpType
    Act = mybir.ActivationFunctionType
    X = mybir.AxisListType.X

    J2, Kpos2 = p.J2, p.Kpos2
    NCLS = len(p.n)
    NCH = D // 128

    nc = bacc.Bacc("TRN2", target_bir_lowering=False, debug=False, num_devices=NCORES)

    xT = nc.dram_tensor("xT", [D, B], bf16, kind="ExternalInput").ap()
    xaT = nc.dram_tensor("xaT", [D, MA], bf16, kind="ExternalInput").ap()
    xa = nc.dram_tensor("xa", [MA, D], bf16, kind="ExternalInput").ap()
    pm7 = nc.dram_tensor("pm7", [NCLS, MA, Kpos2], i8, kind="ExternalInput").ap()
    nm = nc.dram_tensor("nm", [MA, B], i8, kind="ExternalInput").ap()
    out = nc.dram_tensor("out", [1, 2], f32, kind="ExternalOutput").ap()

    with tile.TileContext(nc) as tc, ExitStack() as ctx:
        pool = ctx.enter_context(tc.tile_pool(name="sb", bufs=1))
        sqpool = ctx.enter_context(tc.tile_pool(name="sq", bufs=3))
        scrA = ctx.enter_context(tc.tile_pool(name="scrA", bufs=4))
        scrV = ctx.enter_context(tc.tile_pool(name="scrV", bufs=4))
        pp = ctx.enter_context(tc.tile_pool(name="ps", bufs=1, space="PSUM"))

        ones_bf = pool.tile([128, 1], bf16)
        nc.gpsimd.memset(ones_bf[:], 1.0)
        ones_f32 = pool.tile([128, 1], f32)
        nc.gpsimd.memset(ones_f32[:], 1.0)
        ones_row = pool.tile([1, MA], f32)
        nc.gpsimd.memset(ones_row[:], 1.0)

        # ---- loads (per-chunk so squares/matmuls pipeline) --------------
        xTv = xT.rearrange("(c p) j -> p c j", p=128)
        xT_t = pool.tile([128, NCH, B], bf16)
        for q in range(NCH):
            nc.sync.dma_start(xT_t[:, q, :], xTv[:, q, :])
        xaTv = xaT.rearrange("(c p) j -> p c j", p=128)
        xaT_t = pool.tile([128, NCH, MA], bf16)
        nc.sync.dma_start(xaT_t[:], xaTv)
        xa_t = pool.tile([MA, D], bf16)
        nc.sync.dma_start(xa_t[:], xa)
        pm7_t = pool.tile([MA, NCLS, Kpos2], i8)
        nc.sync.dma_start(pm7_t[:], pm7.rearrange("k m q -> m k q"))
        nm_t = pool.tile([MA, B], i8)
        nc.sync.dma_start(nm_t[:], nm)

        # ---- column norms ssq[j] = sum_d x[d,j]^2 -----------------------
        ps_ssq = pp.tile([1, B], f32)
        for q in range(NCH):
            sq = sqpool.tile([128, B], bf16, tag="sq")
            nc.scalar.activation(sq[:], xT_t[:, q, :], Act.Square)
            nc.tensor.matmul(
                ps_ssq[:], ones_bf[:], sq[:], start=(q == 0), stop=(q == NCH - 1)
            )
        nrm = pool.tile([1, B], f32)
        nc.scalar.activation(nrm[:], ps_ssq[:], Act.Sqrt)
        invn = pool.tile([1, B], f32)
        nc.vector.reciprocal(invn[:], nrm[:])

        # ---- anchor norms ----------------------------------------------
        scr_a = pool.tile([MA, D], bf16)
        ssqa = pool.tile([MA, 1], f32)
        nc.scalar.activation(scr_a[:], xa_t[:], Act.Square, accum_out=ssqa[:])
        nrma = pool.tile([MA, 1], f32)
        nc.scalar.activation(nrma[:], ssqa[:], Act.Sqrt)
        invna = pool.tile([MA, 1], f32)
        nc.vector.reciprocal(invna[:], nrma[:])

        # ---- S = G*invna*invn (the "1 -" of cosine distance cancels in
        # d_ij - d_ik, so we work with similarities directly:
        # t = d_ij - d_ik + m = (m - S_ij) + S_ik) ------------------------
        ps_G = pp.tile([MA, B], f32)
        for q in range(NCH):
            nc.tensor.matmul(
                ps_G[:], xaT_t[:, q, :], xT_t[:, q, :],
                start=(q == 0), stop=(q == NCH - 1),
            )
        ps_B = pp.tile([MA, B], f32)
        nc.tensor.matmul(ps_B[:], ones_row[:], invn[:], start=True, stop=True)
        invnB = pool.tile([MA, B], f32)
        nc.scalar.activation(invnB[:], ps_B[:], Act.Copy)
        Sm = pool.tile([MA, B], bf16)
        nc.vector.scalar_tensor_tensor(
            Sm[:], ps_G[:], invna[:], invnB[:], Alu.mult, Alu.mult
        )
        ms = pool.tile([MA, B], f32)
        nc.vector.tensor_scalar(ms[:], Sm[:], -1.0, MARGIN, Alu.mult, Alu.add)

        # ---- POS bias = m - S_ij (compacted, data-driven classes) -------
        posf = pool.tile([MA, Kpos2], f32)
        nc.gpsimd.memset(posf[:], -BIG)
        for i in range(NCLS):
            s, nk = p.starts[i], p.n[i]
            nc.vector.copy_predicated(
                posf[:, 0:nk], pm7_t[:, i, 0:nk], ms[:, s : s + nk]
            )
        POSst = pool.tile([128, J2], f32)
        nc.gpsimd.memset(POSst[:], -BIG)
        pe = posf.rearrange("p (a two) -> p two a", two=2)
        nc.vector.tensor_copy(POSst[0:MA, :], pe[:, 0, :])
        nc.sync.dma_start(POSst[64 : 64 + MA, :], pe[:, 1, :])

        # ---- NEG = S_ik (dense bf16; same-class columns -> -BIG) --------
        NEGS = pool.tile([128, B], bf16)
        nc.gpsimd.memset(NEGS[:], -BIG)
        nc.vector.copy_predicated(NEGS[0:MA, :], nm_t[:], Sm[:])
        nc.sync.dma_start(NEGS[64 : 64 + MA, :], NEGS[0:MA, :])

        # ---- main loop: ACT relu / DVE count, PE reduces both -----------
        ps_sum = pp.tile([1, B], f32)
        ps_cnt = pp.tile([1, B], f32)
        for jj in range(J2):
            sA = scrA.tile([128, B], bf16, tag="sA")
            nc.scalar.activation(
                sA[:], NEGS[:], Act.Relu, bias=POSst[:, jj : jj + 1]
            )
            nc.tensor.matmul(
                ps_sum[:], ones_bf[:], sA[:],
                start=(jj == 0), stop=(jj == J2 - 1), skip_group_check=True,
            )
            sV = scrV.tile([128, B], bf16, tag="sV")
            nc.vector.tensor_scalar(
                sV[:], NEGS[:], POSst[:, jj : jj + 1], 0.0, Alu.add, Alu.is_gt
            )
            nc.tensor.matmul(
                ps_cnt[:], ones_bf[:], sV[:],
                start=(jj == 0), stop=(jj == J2 - 1), skip_group_check=True,
            )

        # ---- final reduction --------------------------------------------
        outs = pool.tile([1, 2], f32)
        nc.vector.tensor_reduce(outs[:, 0:1], ps_sum[:], X, Alu.add)
        nc.vector.tensor_reduce(outs[:, 1:2], ps_cnt[:], X, Alu.add)
        nc.sync.dma_start(out, outs[:])

    nc.compile()
    return nc


def _in_maps(p: Plan, emb: np.ndarray):
    import ml_dtypes

    bf = ml_dtypes.bfloat16
    xs = np.ascontiguousarray(emb[p.order])
    xT = np.ascontiguousarray(xs.T.astype(bf))
    maps = []
    for c in range(NCORES):
        xa = xs[MA * c : MA * (c + 1)]
        maps.append(
            {
                "xT": xT,
                "xaT": np.ascontiguousarray(xa.T.astype(bf)),
                "xa": np.ascontiguousarray(xa.astype(bf)),
                "pm7": p.pm7[c],
                "nm": p.negmask[c],
            }
        )
    return maps


LAST_RESULT = None  # BassKernelResults of the most recent run (for profiling)


def kernel(embeddings, labels):
    global LAST_RESULT
    import os

    from concourse.bass_utils import run_bass_kernel_spmd

    emb = np.ascontiguousarray(np.asarray(embeddings, dtype=np.float32))
    lab = np.asarray(labels).astype(np.int64)
    p = _make_plan(lab)
    if p.key not in _PROG_CACHE:
        _PROG_CACHE[p.key] = _build_program(p)
    nc = _PROG_CACHE[p.key]
    trace = bool(int(os.environ.get("TRIPLET_TRACE", "0")))
    kw = {}
    if os.environ.get("TRIPLET_TMPDIR"):
        kw["tmpdir"] = os.environ["TRIPLET_TMPDIR"]
    LAST_RESULT = run_bass_kernel_spmd(
        nc, _in_maps(p, emb), list(range(NCORES)), trace=trace, **kw
    )
    res = LAST_RESULT.results
    S = 0.0
    C = 0.0
    for r in res:
        o = np.asarray(r["out"], dtype=np.float64).reshape(-1)
        S += o[0]
        C += o[1]
    return np.float32(S / (C + EPS))
